# revision 8
# baseline (speedup 1.0000x reference)
"""MoE layer (E=8, top-2) on 8 NeuronCores via Bass/Tile.

Strategy: 4 token-groups x 2 expert-groups.
  Core c = (g, h), g = c // 2 in 0..3, h = c % 2.
  Core (g, h) holds tokens [512*g, 512*(g+1)) and experts [4h, 4h+4).
  Each core computes the full router (all 8 experts, gate rows host-permuted
  so the core's own 4 experts come first -- softmax/top-k are permutation
  equivariant), then the 4 local experts' MLPs densely over its 512 tokens,
  scaled by the top-2 combine weights (zero for non-selected pairs), with
  Sum_e accumulated in PSUM.  Host unshard: out[g] = (outT[g,0] + outT[g,1]).T

  Activations are kept transposed on device (hidden dim on partitions) so all
  matmuls consume natural-layout weights.  Host supplies x already transposed
  per-shard (layout choice of the sharding).  W1/W2 are cast to bf16 on host
  (PE runs bf16 at 1 cyc/row vs fp32 4 cyc/row); accumulation stays fp32 in
  PSUM.  Router runs fully in fp32.
"""

import numpy as np
import ml_dtypes

# Problem shapes (hardcoded per the task contract).
B, S, H, F, E = 2, 1024, 512, 2048, 8
T = B * S              # 2048 tokens
N_CORES = 8
TG, EG = 4, 2          # token groups x expert groups
T_C = T // TG          # 512 tokens per core
E_LOC = E // EG        # 4 experts per core
HC = H // 128          # 4
FC = F // 128          # 16
TT = T_C // 128        # 4

_cache = {}


def _build_bass():
    import concourse.mybir as mybir
    import concourse.tile as tile
    from concourse import bacc

    f32 = mybir.dt.float32
    bf16 = mybir.dt.bfloat16

    nc = bacc.Bacc(None, target_bir_lowering=False, debug=False)
    with tile.TileContext(nc) as tc:
        with tc.tile_pool(name="dram", bufs=1, space="DRAM") as dram:
            xT_d = dram.tile([H, T_C], f32, kind="ExternalInput", name="xT", uniquify=False)
            wgT_d = dram.tile([H, E], f32, kind="ExternalInput", name="wgT", uniquify=False)
            w1_d = dram.tile([E_LOC, H, F], bf16, kind="ExternalInput", name="w1", uniquify=False)
            b1t_d = dram.tile([128, FC * E_LOC], f32, kind="ExternalInput", name="b1t", uniquify=False)
            w2_d = dram.tile([E_LOC, F, H], bf16, kind="ExternalInput", name="w2", uniquify=False)
            b2_d = dram.tile([E_LOC, H], f32, kind="ExternalInput", name="b2", uniquify=False)
            ind_d = dram.tile([E_LOC, E_LOC * 128], f32, kind="ExternalInput", name="ind", uniquify=False)
            outT_d = dram.tile([H, T_C], f32, kind="ExternalOutput", name="outT", uniquify=False)
            _moe_body(nc, tc, mybir, xT_d, wgT_d, w1_d, b1t_d, w2_d, b2_d, ind_d, outT_d)
    nc.compile()
    return nc


def _moe_body(nc, tc, mybir, xT_d, wgT_d, w1_d, b1t_d, w2_d, b2_d, ind_d, outT_d):
    from concourse.masks import make_identity

    f32 = mybir.dt.float32
    bf16 = mybir.dt.bfloat16
    ALU = mybir.AluOpType
    ACTF = mybir.ActivationFunctionType
    AXIS = mybir.AxisListType

    with (
        tc.tile_pool(name="constp", bufs=1) as constp,
        tc.tile_pool(name="xp", bufs=1) as xp,
        tc.tile_pool(name="w1p", bufs=2) as w1p,
        tc.tile_pool(name="w2p", bufs=2) as w2p,
        tc.tile_pool(name="actp", bufs=3) as actp,
        tc.tile_pool(name="rp", bufs=2) as rp,
        tc.tile_pool(name="php", bufs=2, space="PSUM") as php,
        tc.tile_pool(name="pop", bufs=1, space="PSUM") as pop,
        tc.tile_pool(name="pmp", bufs=2, space="PSUM") as pmp,
    ):
        # ---- constants & input loads ----
        identity = constp.tile([128, 128], f32, name="identity")
        make_identity(nc, identity)
        # ind[k, e*128 + m] = (k == e): selects+broadcasts row e of combT via matmul
        ind = constp.tile([E_LOC, E_LOC * 128], f32, name="ind")
        nc.sync.dma_start(out=ind, in_=ind_d[:, :])

        xT = []
        xTb = []
        for hc in range(HC):
            t = xp.tile([128, T_C], f32, name=f"xTf{hc}", tag=f"xTf{hc}")
            nc.sync.dma_start(out=t, in_=xT_d[hc * 128:(hc + 1) * 128, :])
            xT.append(t)
        wgT = []
        for hc in range(HC):
            t = xp.tile([128, E], f32, name=f"wgT{hc}", tag=f"wgT{hc}")
            nc.sync.dma_start(out=t, in_=wgT_d[hc * 128:(hc + 1) * 128, :])
            wgT.append(t)
        b1t = xp.tile([128, FC * E_LOC], f32, name="b1t", tag="b1t")
        nc.sync.dma_start(out=b1t, in_=b1t_d[:, :])
        b2 = xp.tile([E_LOC, H], f32, name="b2", tag="b2")
        nc.sync.dma_start(out=b2, in_=b2_d[:, :])
        for hc in range(HC):
            tb = xp.tile([128, T_C], bf16, name=f"xTb{hc}", tag=f"xTb{hc}")
            nc.vector.tensor_copy(out=tb, in_=xT[hc])
            xTb.append(tb)

        # ---- router: scores -> top-2 renormalized combine weights ----
        combT_f = xp.tile([E_LOC, T_C], f32, name="combT_f", tag="combT_f")
        for tt in range(TT):
            tsl = slice(tt * 128, (tt + 1) * 128)
            ps = pmp.tile([128, E], f32, name=f"ps{tt}", tag="pm")
            for hc in range(HC):
                nc.tensor.matmul(
                    out=ps, lhsT=xT[hc][:, tsl], rhs=wgT[hc],
                    start=(hc == 0), stop=(hc == HC - 1),
                )
            s = rp.tile([128, E], f32, name=f"s{tt}", tag="s")
            nc.vector.tensor_copy(out=s, in_=ps)
            m1 = rp.tile([128, 1], f32, name=f"m1{tt}", tag="m1")
            nc.vector.tensor_reduce(out=m1, in_=s, axis=AXIS.X, op=ALU.max)
            is1 = rp.tile([128, E], f32, name=f"is1{tt}", tag="is1")
            nc.vector.tensor_scalar(out=is1, in0=s, scalar1=m1, scalar2=None, op0=ALU.is_ge)
            s2 = rp.tile([128, E], f32, name=f"s2{tt}", tag="s2")
            nc.vector.scalar_tensor_tensor(
                out=s2, in0=is1, scalar=-1e30, in1=s, op0=ALU.mult, op1=ALU.add,
            )
            m2 = rp.tile([128, 1], f32, name=f"m2{tt}", tag="m2")
            nc.vector.tensor_reduce(out=m2, in_=s2, axis=AXIS.X, op=ALU.max)
            is2 = rp.tile([128, E], f32, name=f"is2{tt}", tag="is2")
            nc.vector.tensor_scalar(out=is2, in0=s2, scalar1=m2, scalar2=None, op0=ALU.is_ge)
            dm = rp.tile([128, 1], f32, name=f"dm{tt}", tag="dm")
            nc.vector.tensor_sub(dm, m2, m1)
            w2s = rp.tile([128, 1], f32, name=f"w2s{tt}", tag="w2s")
            nc.scalar.activation(out=w2s, in_=dm, func=ACTF.Sigmoid)
            # comb = is1 * (1 - w2s) + is2 * w2s
            w1s = rp.tile([128, 1], f32, name=f"w1s{tt}", tag="w1s")
            nc.scalar.activation(out=w1s, in_=w2s, func=ACTF.Identity, bias=1.0, scale=-1.0)
            comb1 = rp.tile([128, E], f32, name=f"comb1{tt}", tag="comb1")
            nc.vector.tensor_scalar(out=comb1, in0=is1, scalar1=w1s, scalar2=None, op0=ALU.mult)
            comb = rp.tile([128, E], f32, name=f"comb{tt}", tag="comb")
            nc.vector.scalar_tensor_tensor(
                out=comb, in0=is2, scalar=w2s, in1=comb1, op0=ALU.mult, op1=ALU.add,
            )
            # transpose [128, E] -> [E, 128]; keep local-expert rows
            pst = pmp.tile([E, 128], f32, name=f"pst{tt}", tag="pm")
            nc.tensor.transpose(out=pst, in_=comb, identity=identity[:, :])
            nc.vector.tensor_copy(out=combT_f[:, tsl], in_=pst[0:E_LOC, :])

        # ---- output accumulators; weighted b2 bias via K=4 matmul ----
        out_ps = []
        for hc in range(HC):
            t = pop.tile([128, T_C], f32, name=f"outp{hc}", tag=f"outp{hc}")
            out_ps.append(t)
            nc.tensor.matmul(
                out=t, lhsT=b2[0:E_LOC, hc * 128:(hc + 1) * 128], rhs=combT_f[:, :],
                start=True, stop=False,
            )

        # ---- main loop over local experts ----
        for e in range(E_LOC):
            w1t = []
            for hc in range(HC):
                t = w1p.tile([128, F], bf16, name=f"w1_{e}_{hc}", tag=f"w1_{hc}")
                nc.sync.dma_start(out=t, in_=w1_d[e, hc * 128:(hc + 1) * 128, :])
                w1t.append(t)
            w2t = []
            for fc in range(FC):
                t = w2p.tile([128, H], bf16, name=f"w2_{e}_{fc}", tag=f"w2_{fc}")
                nc.sync.dma_start(out=t, in_=w2_d[e, fc * 128:(fc + 1) * 128, :])
                w2t.append(t)

            # broadcast this expert's combine row across 128 partitions
            cb_ps = pmp.tile([128, T_C], f32, name=f"cbp{e}", tag="pm")
            nc.tensor.matmul(
                out=cb_ps, lhsT=ind[:, e * 128:(e + 1) * 128], rhs=combT_f[:, :],
                start=True, stop=True,
            )
            combB = actp.tile([128, T_C], bf16, name=f"combB{e}", tag="combB", bufs=2)
            nc.vector.tensor_copy(out=combB, in_=cb_ps)

            for fc in range(FC):
                fsl = slice(fc * 128, (fc + 1) * 128)
                hps = php.tile([128, T_C], f32, name=f"h{e}_{fc}", tag="h")
                for hc in range(HC):
                    nc.tensor.matmul(
                        out=hps, lhsT=w1t[hc][:, fsl], rhs=xTb[hc],
                        start=(hc == 0), stop=(hc == HC - 1),
                    )
                asil = actp.tile([128, T_C], bf16, name=f"as{e}_{fc}", tag="asil")
                nc.scalar.activation(
                    out=asil, in_=hps, func=ACTF.Silu,
                    bias=b1t[:, fc * E_LOC + e: fc * E_LOC + e + 1], scale=1.0,
                )
                asc = actp.tile([128, T_C], bf16, name=f"ac{e}_{fc}", tag="asc")
                nc.vector.tensor_mul(asc, asil, combB)
                for hc in range(HC):
                    nc.tensor.matmul(
                        out=out_ps[hc], lhsT=w2t[fc][:, hc * 128:(hc + 1) * 128], rhs=asc,
                        start=False, stop=(e == E_LOC - 1 and fc == FC - 1),
                    )

        # ---- epilogue: PSUM -> SBUF -> DRAM ----
        for hc in range(HC):
            ot = actp.tile([128, T_C], f32, name=f"ot{hc}", tag="ot")
            nc.vector.tensor_copy(out=ot, in_=out_ps[hc])
            nc.sync.dma_start(out=outT_d[hc * 128:(hc + 1) * 128, :], in_=ot)


def _get_nc():
    if "nc" not in _cache:
        _cache["nc"] = _build_bass()
    return _cache["nc"]


def _make_in_maps(x, Wg, W1, b1, W2, b2):
    xf = np.ascontiguousarray(x.reshape(T, H), dtype=np.float32)
    in_maps = []
    for c in range(N_CORES):
        g, h = divmod(c, 2)
        el = slice(E_LOC * h, E_LOC * (h + 1))
        perm = list(range(E_LOC * h, E_LOC * (h + 1))) + \
               [i for i in range(E) if not (E_LOC * h <= i < E_LOC * (h + 1))]
        xTc = np.ascontiguousarray(xf[g * T_C:(g + 1) * T_C].T)
        wgTc = np.ascontiguousarray(Wg[perm].T.astype(np.float32))
        w1c = np.ascontiguousarray(W1[el]).astype(ml_dtypes.bfloat16)
        w2c = np.ascontiguousarray(W2[el]).astype(ml_dtypes.bfloat16)
        b1h = np.asarray(b1[el], dtype=np.float32)
        b1tc = np.ascontiguousarray(
            b1h.reshape(E_LOC, FC, 128).transpose(2, 1, 0).reshape(128, FC * E_LOC))
        b2c = np.ascontiguousarray(b2[el], dtype=np.float32)
        indc = np.kron(np.eye(E_LOC, dtype=np.float32), np.ones((1, 128), np.float32))
        in_maps.append({
            "xT": xTc, "wgT": wgTc, "w1": w1c, "b1t": b1tc, "w2": w2c, "b2": b2c,
            "ind": indc,
        })
    return in_maps


def kernel(x, Wg, W1, b1, W2, b2, _trace=False, _trace_kwargs=None):
    from concourse.bass_utils import run_bass_kernel_spmd

    nc = _get_nc()
    in_maps = _make_in_maps(
        np.asarray(x, np.float32), np.asarray(Wg, np.float32),
        np.asarray(W1, np.float32), np.asarray(b1, np.float32),
        np.asarray(W2, np.float32), np.asarray(b2, np.float32))
    kw = {}
    if _trace:
        kw.update(trace=True, **(_trace_kwargs or {}))
    res = run_bass_kernel_spmd(nc, in_maps, core_ids=list(range(N_CORES)), **kw)
    _cache["last_results"] = res
    outs = [r["outT"] for r in res.results]
    of = np.empty((T, H), np.float32)
    for g in range(TG):
        of[g * T_C:(g + 1) * T_C] = (outs[2 * g] + outs[2 * g + 1]).T
    return of.reshape(B, S, H)


# revision 9
# speedup vs baseline: 1.2962x; 1.2962x over previous
"""MoE layer (E=8, top-2) on 8 NeuronCores via Bass/Tile.

Strategy: 4 token-groups x 2 expert-groups.
  Core c = (g, h), g = c // 2 in 0..3, h = c % 2.
  Core (g, h) holds tokens [512*g, 512*(g+1)) and experts [4h, 4h+4).
  Each core computes the full router (all 8 experts, gate rows host-permuted
  so the core's own 4 experts come first -- softmax/top-k are permutation
  equivariant), then the 4 local experts' MLPs densely over its 512 tokens,
  scaled by the top-2 combine weights (zero for non-selected pairs), with
  Sum_e accumulated in PSUM.  Host unshard: out[g] = (outT[g,0] + outT[g,1]).T

  Activations are kept transposed on device (hidden dim on partitions) so all
  matmuls consume natural-layout weights.  Host supplies x already transposed
  per-shard (layout choice of the sharding).  W1/W2 are cast to bf16 on host
  (PE runs bf16 at 1 cyc/row vs fp32 4 cyc/row); accumulation stays fp32 in
  PSUM.  Router runs fully in fp32.
"""

import numpy as np
import ml_dtypes

# Problem shapes (hardcoded per the task contract).
B, S, H, F, E = 2, 1024, 512, 2048, 8
T = B * S              # 2048 tokens
N_CORES = 8
TG, EG = 4, 2          # token groups x expert groups
T_C = T // TG          # 512 tokens per core
E_LOC = E // EG        # 4 experts per core
HC = H // 128          # 4
FC = F // 128          # 16
TT = T_C // 128        # 4

_cache = {}


def _build_bass():
    import concourse.mybir as mybir
    import concourse.tile as tile
    from concourse import bacc

    f32 = mybir.dt.float32
    bf16 = mybir.dt.bfloat16

    nc = bacc.Bacc(None, target_bir_lowering=False, debug=False)
    with tile.TileContext(nc) as tc:
        with tc.tile_pool(name="dram", bufs=1, space="DRAM") as dram:
            xT_d = dram.tile([H, T_C], f32, kind="ExternalInput", name="xT", uniquify=False)
            wgT_d = dram.tile([H, E], f32, kind="ExternalInput", name="wgT", uniquify=False)
            w1_d = dram.tile([E_LOC, H, F], bf16, kind="ExternalInput", name="w1", uniquify=False)
            b1t_d = dram.tile([128, FC * E_LOC], f32, kind="ExternalInput", name="b1t", uniquify=False)
            w2_d = dram.tile([E_LOC, F, H], bf16, kind="ExternalInput", name="w2", uniquify=False)
            b2_d = dram.tile([E_LOC, H], f32, kind="ExternalInput", name="b2", uniquify=False)
            ind_d = dram.tile([E_LOC, E_LOC * 128], f32, kind="ExternalInput", name="ind", uniquify=False)
            outT_d = dram.tile([H, T_C], f32, kind="ExternalOutput", name="outT", uniquify=False)
            _moe_body(nc, tc, mybir, xT_d, wgT_d, w1_d, b1t_d, w2_d, b2_d, ind_d, outT_d)
    nc.compile()
    return nc


def _moe_body(nc, tc, mybir, xT_d, wgT_d, w1_d, b1t_d, w2_d, b2_d, ind_d, outT_d):
    from concourse.masks import make_identity

    f32 = mybir.dt.float32
    bf16 = mybir.dt.bfloat16
    ALU = mybir.AluOpType
    ACTF = mybir.ActivationFunctionType
    AXIS = mybir.AxisListType

    with (
        tc.tile_pool(name="constp", bufs=1) as constp,
        tc.tile_pool(name="xp", bufs=1) as xp,
        tc.tile_pool(name="w1p", bufs=2) as w1p,
        tc.tile_pool(name="w2p", bufs=2) as w2p,
        tc.tile_pool(name="actp", bufs=3) as actp,
        tc.tile_pool(name="rp", bufs=2) as rp,
        tc.tile_pool(name="php", bufs=2, space="PSUM") as php,
        tc.tile_pool(name="pop", bufs=1, space="PSUM") as pop,
        tc.tile_pool(name="pmp", bufs=2, space="PSUM") as pmp,
    ):
        # ---- constants & input loads ----
        identity = constp.tile([128, 128], f32, name="identity")
        make_identity(nc, identity)
        # ind[k, e*128 + m] = (k == e): selects+broadcasts row e of combT via matmul
        ind = constp.tile([E_LOC, E_LOC * 128], f32, name="ind")
        nc.sync.dma_start(out=ind, in_=ind_d[:, :])

        xT = []
        xTb = []
        for hc in range(HC):
            t = xp.tile([128, T_C], f32, name=f"xTf{hc}", tag=f"xTf{hc}")
            nc.sync.dma_start(out=t, in_=xT_d[hc * 128:(hc + 1) * 128, :])
            xT.append(t)
        wgT = []
        for hc in range(HC):
            t = xp.tile([128, E], f32, name=f"wgT{hc}", tag=f"wgT{hc}")
            nc.sync.dma_start(out=t, in_=wgT_d[hc * 128:(hc + 1) * 128, :])
            wgT.append(t)
        b1t = xp.tile([128, FC * E_LOC], f32, name="b1t", tag="b1t")
        nc.sync.dma_start(out=b1t, in_=b1t_d[:, :])
        b2 = xp.tile([E_LOC, H], f32, name="b2", tag="b2")
        nc.sync.dma_start(out=b2, in_=b2_d[:, :])
        for hc in range(HC):
            tb = xp.tile([128, T_C], bf16, name=f"xTb{hc}", tag=f"xTb{hc}")
            nc.vector.tensor_copy(out=tb, in_=xT[hc])
            xTb.append(tb)

        # ---- router: scores -> top-2 renormalized combine weights ----
        combT_f = xp.tile([E_LOC, T_C], f32, name="combT_f", tag="combT_f")
        for tt in range(TT):
            tsl = slice(tt * 128, (tt + 1) * 128)
            ps = pmp.tile([128, E], f32, name=f"ps{tt}", tag="pm")
            for hc in range(HC):
                nc.tensor.matmul(
                    out=ps, lhsT=xT[hc][:, tsl], rhs=wgT[hc],
                    start=(hc == 0), stop=(hc == HC - 1),
                )
            s = rp.tile([128, E], f32, name=f"s{tt}", tag="s")
            nc.vector.tensor_copy(out=s, in_=ps)
            m1 = rp.tile([128, 1], f32, name=f"m1{tt}", tag="m1")
            nc.vector.tensor_reduce(out=m1, in_=s, axis=AXIS.X, op=ALU.max)
            is1 = rp.tile([128, E], f32, name=f"is1{tt}", tag="is1")
            nc.vector.tensor_scalar(out=is1, in0=s, scalar1=m1, scalar2=None, op0=ALU.is_ge)
            s2 = rp.tile([128, E], f32, name=f"s2{tt}", tag="s2")
            nc.vector.scalar_tensor_tensor(
                out=s2, in0=is1, scalar=-1e30, in1=s, op0=ALU.mult, op1=ALU.add,
            )
            m2 = rp.tile([128, 1], f32, name=f"m2{tt}", tag="m2")
            nc.vector.tensor_reduce(out=m2, in_=s2, axis=AXIS.X, op=ALU.max)
            is2 = rp.tile([128, E], f32, name=f"is2{tt}", tag="is2")
            nc.vector.tensor_scalar(out=is2, in0=s2, scalar1=m2, scalar2=None, op0=ALU.is_ge)
            dm = rp.tile([128, 1], f32, name=f"dm{tt}", tag="dm")
            nc.vector.tensor_sub(dm, m2, m1)
            w2s = rp.tile([128, 1], f32, name=f"w2s{tt}", tag="w2s")
            nc.scalar.activation(out=w2s, in_=dm, func=ACTF.Sigmoid)
            # comb = is1 * (1 - w2s) + is2 * w2s
            w1s = rp.tile([128, 1], f32, name=f"w1s{tt}", tag="w1s")
            nc.scalar.activation(out=w1s, in_=w2s, func=ACTF.Identity, bias=1.0, scale=-1.0)
            comb1 = rp.tile([128, E], f32, name=f"comb1{tt}", tag="comb1")
            nc.vector.tensor_scalar(out=comb1, in0=is1, scalar1=w1s, scalar2=None, op0=ALU.mult)
            comb = rp.tile([128, E], f32, name=f"comb{tt}", tag="comb")
            nc.vector.scalar_tensor_tensor(
                out=comb, in0=is2, scalar=w2s, in1=comb1, op0=ALU.mult, op1=ALU.add,
            )
            # transpose [128, E] -> [E, 128]; keep local-expert rows
            pst = pmp.tile([E, 128], f32, name=f"pst{tt}", tag="pm")
            nc.tensor.transpose(out=pst, in_=comb, identity=identity[:, :])
            nc.vector.tensor_copy(out=combT_f[:, tsl], in_=pst[0:E_LOC, :])

        # ---- output accumulators; weighted b2 bias via K=4 matmul ----
        out_ps = []
        for hc in range(HC):
            t = pop.tile([128, T_C], f32, name=f"outp{hc}", tag=f"outp{hc}")
            out_ps.append(t)
            nc.tensor.matmul(
                out=t, lhsT=b2[0:E_LOC, hc * 128:(hc + 1) * 128], rhs=combT_f[:, :],
                start=True, stop=False,
            )

        # ---- main loop over local experts ----
        for e in range(E_LOC):
            w1t = []
            for hc in range(HC):
                t = w1p.tile([128, F], bf16, name=f"w1_{e}_{hc}", tag=f"w1_{hc}")
                nc.sync.dma_start(out=t, in_=w1_d[e, hc * 128:(hc + 1) * 128, :])
                w1t.append(t)
            w2t = []
            for fc in range(FC):
                t = w2p.tile([128, H], bf16, name=f"w2_{e}_{fc}", tag=f"w2_{fc}")
                nc.scalar.dma_start(out=t, in_=w2_d[e, fc * 128:(fc + 1) * 128, :])
                w2t.append(t)

            # broadcast this expert's combine row across 128 partitions
            cb_ps = pmp.tile([128, T_C], f32, name=f"cbp{e}", tag="pm")
            nc.tensor.matmul(
                out=cb_ps, lhsT=ind[:, e * 128:(e + 1) * 128], rhs=combT_f[:, :],
                start=True, stop=True,
            )
            combB = actp.tile([128, T_C], bf16, name=f"combB{e}", tag="combB", bufs=2)
            nc.vector.tensor_copy(out=combB, in_=cb_ps)

            for fc in range(FC):
                fsl = slice(fc * 128, (fc + 1) * 128)
                hps = php.tile([128, T_C], f32, name=f"h{e}_{fc}", tag="h")
                for hc in range(HC):
                    nc.tensor.matmul(
                        out=hps, lhsT=w1t[hc][:, fsl], rhs=xTb[hc],
                        start=(hc == 0), stop=(hc == HC - 1),
                    )
                asil = actp.tile([128, T_C], bf16, name=f"as{e}_{fc}", tag="asil")
                nc.scalar.activation(
                    out=asil, in_=hps, func=ACTF.Silu,
                    bias=b1t[:, fc * E_LOC + e: fc * E_LOC + e + 1], scale=1.0,
                )
                asc = actp.tile([128, T_C], bf16, name=f"ac{e}_{fc}", tag="asc")
                nc.vector.tensor_mul(asc, asil, combB)
                for hc in range(HC):
                    nc.tensor.matmul(
                        out=out_ps[hc], lhsT=w2t[fc][:, hc * 128:(hc + 1) * 128], rhs=asc,
                        start=False, stop=(e == E_LOC - 1 and fc == FC - 1),
                    )

        # ---- epilogue: PSUM -> SBUF -> DRAM ----
        for hc in range(HC):
            ot = actp.tile([128, T_C], f32, name=f"ot{hc}", tag="ot")
            nc.vector.tensor_copy(out=ot, in_=out_ps[hc])
            nc.sync.dma_start(out=outT_d[hc * 128:(hc + 1) * 128, :], in_=ot)


def _get_nc():
    if "nc" not in _cache:
        _cache["nc"] = _build_bass()
    return _cache["nc"]


def _make_in_maps(x, Wg, W1, b1, W2, b2):
    xf = np.ascontiguousarray(x.reshape(T, H), dtype=np.float32)
    in_maps = []
    for c in range(N_CORES):
        g, h = divmod(c, 2)
        el = slice(E_LOC * h, E_LOC * (h + 1))
        perm = list(range(E_LOC * h, E_LOC * (h + 1))) + \
               [i for i in range(E) if not (E_LOC * h <= i < E_LOC * (h + 1))]
        xTc = np.ascontiguousarray(xf[g * T_C:(g + 1) * T_C].T)
        wgTc = np.ascontiguousarray(Wg[perm].T.astype(np.float32))
        w1c = np.ascontiguousarray(W1[el]).astype(ml_dtypes.bfloat16)
        w2c = np.ascontiguousarray(W2[el]).astype(ml_dtypes.bfloat16)
        b1h = np.asarray(b1[el], dtype=np.float32)
        b1tc = np.ascontiguousarray(
            b1h.reshape(E_LOC, FC, 128).transpose(2, 1, 0).reshape(128, FC * E_LOC))
        b2c = np.ascontiguousarray(b2[el], dtype=np.float32)
        indc = np.kron(np.eye(E_LOC, dtype=np.float32), np.ones((1, 128), np.float32))
        in_maps.append({
            "xT": xTc, "wgT": wgTc, "w1": w1c, "b1t": b1tc, "w2": w2c, "b2": b2c,
            "ind": indc,
        })
    return in_maps


def kernel(x, Wg, W1, b1, W2, b2, _trace=False, _trace_kwargs=None):
    from concourse.bass_utils import run_bass_kernel_spmd

    nc = _get_nc()
    in_maps = _make_in_maps(
        np.asarray(x, np.float32), np.asarray(Wg, np.float32),
        np.asarray(W1, np.float32), np.asarray(b1, np.float32),
        np.asarray(W2, np.float32), np.asarray(b2, np.float32))
    kw = {}
    if _trace:
        kw.update(trace=True, **(_trace_kwargs or {}))
    res = run_bass_kernel_spmd(nc, in_maps, core_ids=list(range(N_CORES)), **kw)
    _cache["last_results"] = res
    outs = [r["outT"] for r in res.results]
    of = np.empty((T, H), np.float32)
    for g in range(TG):
        of[g * T_C:(g + 1) * T_C] = (outs[2 * g] + outs[2 * g + 1]).T
    return of.reshape(B, S, H)


# revision 10
# speedup vs baseline: 2.8163x; 2.1728x over previous
"""MoE layer (E=8, top-2) on 8 NeuronCores via Bass/Tile.

Strategy: 4 token-groups x 2 expert-groups.
  Core c = (g, h), g = c // 2 in 0..3, h = c % 2.
  Core (g, h) holds tokens [512*g, 512*(g+1)) and experts [4h, 4h+4).
  Each core computes the full router (all 8 experts, gate rows host-permuted
  so the core's own 4 experts come first -- softmax/top-k are permutation
  equivariant), then the 4 local experts' MLPs densely over its 512 tokens,
  scaled by the top-2 combine weights (zero for non-selected pairs), with
  Sum_e accumulated in PSUM.  Host unshard: out[g] = (outT[g,0] + outT[g,1]).T

  Activations are kept transposed on device (hidden dim on partitions) so all
  matmuls consume natural-layout weights.  Host supplies x already transposed
  per-shard (layout choice of the sharding).  W1/W2 are cast to bf16 on host
  (PE runs bf16 at 1 cyc/row vs fp32 4 cyc/row); accumulation stays fp32 in
  PSUM.  Router runs fully in fp32.
"""

import numpy as np
import ml_dtypes

# Problem shapes (hardcoded per the task contract).
B, S, H, F, E = 2, 1024, 512, 2048, 8
T = B * S              # 2048 tokens
N_CORES = 8
TG, EG = 4, 2          # token groups x expert groups
T_C = T // TG          # 512 tokens per core
E_LOC = E // EG        # 4 experts per core
HC = H // 128          # 4
FC = F // 128          # 16
TT = T_C // 128        # 4

_cache = {}


def _build_bass():
    import concourse.mybir as mybir
    import concourse.tile as tile
    from concourse import bacc

    f32 = mybir.dt.float32
    bf16 = mybir.dt.bfloat16

    nc = bacc.Bacc(None, target_bir_lowering=False, debug=False)
    with tile.TileContext(nc) as tc:
        with tc.tile_pool(name="dram", bufs=1, space="DRAM") as dram:
            xT_d = dram.tile([H, T_C], f32, kind="ExternalInput", name="xT", uniquify=False)
            wgT_d = dram.tile([H, E], f32, kind="ExternalInput", name="wgT", uniquify=False)
            w1_d = dram.tile([E_LOC, H, F], bf16, kind="ExternalInput", name="w1", uniquify=False)
            b1t_d = dram.tile([128, FC * E_LOC], f32, kind="ExternalInput", name="b1t", uniquify=False)
            w2_d = dram.tile([E_LOC, F, H], bf16, kind="ExternalInput", name="w2", uniquify=False)
            b2_d = dram.tile([E_LOC, H], f32, kind="ExternalInput", name="b2", uniquify=False)
            ind_d = dram.tile([E_LOC, E_LOC * 128], f32, kind="ExternalInput", name="ind", uniquify=False)
            outT_d = dram.tile([H, T_C], f32, kind="ExternalOutput", name="outT", uniquify=False)
            _moe_body(nc, tc, mybir, xT_d, wgT_d, w1_d, b1t_d, w2_d, b2_d, ind_d, outT_d)
    nc.compile()
    return nc


def _moe_body(nc, tc, mybir, xT_d, wgT_d, w1_d, b1t_d, w2_d, b2_d, ind_d, outT_d):
    from concourse.masks import make_identity

    f32 = mybir.dt.float32
    bf16 = mybir.dt.bfloat16
    ALU = mybir.AluOpType
    ACTF = mybir.ActivationFunctionType
    AXIS = mybir.AxisListType

    with (
        tc.tile_pool(name="constp", bufs=1) as constp,
        tc.tile_pool(name="xp", bufs=1) as xp,
        tc.tile_pool(name="w1p", bufs=2) as w1p,
        tc.tile_pool(name="w2p", bufs=2) as w2p,
        tc.tile_pool(name="actp", bufs=3) as actp,
        tc.tile_pool(name="rp", bufs=2) as rp,
        tc.tile_pool(name="php", bufs=2, space="PSUM") as php,
        tc.tile_pool(name="pop", bufs=1, space="PSUM") as pop,
        tc.tile_pool(name="pmp", bufs=2, space="PSUM") as pmp,
    ):
        # ---- constants & input loads ----
        identity = constp.tile([128, 128], f32, name="identity")
        make_identity(nc, identity)
        # ind[k, e*128 + m] = (k == e): selects+broadcasts row e of combT via matmul
        ind = constp.tile([E_LOC, E_LOC * 128], f32, name="ind")
        nc.sync.dma_start(out=ind, in_=ind_d[:, :])

        xT = []
        xTb = []
        for hc in range(HC):
            t = xp.tile([128, T_C], f32, name=f"xTf{hc}", tag=f"xTf{hc}")
            nc.sync.dma_start(out=t, in_=xT_d[hc * 128:(hc + 1) * 128, :])
            xT.append(t)
        wgT = []
        for hc in range(HC):
            t = xp.tile([128, E], f32, name=f"wgT{hc}", tag=f"wgT{hc}")
            nc.sync.dma_start(out=t, in_=wgT_d[hc * 128:(hc + 1) * 128, :])
            wgT.append(t)
        b1t = xp.tile([128, FC * E_LOC], f32, name="b1t", tag="b1t")
        nc.sync.dma_start(out=b1t, in_=b1t_d[:, :])
        b2 = xp.tile([E_LOC, H], f32, name="b2", tag="b2")
        nc.sync.dma_start(out=b2, in_=b2_d[:, :])
        for hc in range(HC):
            tb = xp.tile([128, T_C], bf16, name=f"xTb{hc}", tag=f"xTb{hc}")
            nc.vector.tensor_copy(out=tb, in_=xT[hc])
            xTb.append(tb)

        # ---- router: scores -> top-2 renormalized combine weights ----
        combT_f = xp.tile([E_LOC, T_C], f32, name="combT_f", tag="combT_f")
        for tt in range(TT):
            tsl = slice(tt * 128, (tt + 1) * 128)
            ps = pmp.tile([128, E], f32, name=f"ps{tt}", tag="pm")
            for hc in range(HC):
                nc.tensor.matmul(
                    out=ps, lhsT=xT[hc][:, tsl], rhs=wgT[hc],
                    start=(hc == 0), stop=(hc == HC - 1),
                )
            s = rp.tile([128, E], f32, name=f"s{tt}", tag="s")
            nc.vector.tensor_copy(out=s, in_=ps)
            m1 = rp.tile([128, 1], f32, name=f"m1{tt}", tag="m1")
            nc.vector.tensor_reduce(out=m1, in_=s, axis=AXIS.X, op=ALU.max)
            is1 = rp.tile([128, E], f32, name=f"is1{tt}", tag="is1")
            nc.vector.tensor_scalar(out=is1, in0=s, scalar1=m1, scalar2=None, op0=ALU.is_ge)
            s2 = rp.tile([128, E], f32, name=f"s2{tt}", tag="s2")
            nc.vector.scalar_tensor_tensor(
                out=s2, in0=is1, scalar=-1e30, in1=s, op0=ALU.mult, op1=ALU.add,
            )
            m2 = rp.tile([128, 1], f32, name=f"m2{tt}", tag="m2")
            nc.vector.tensor_reduce(out=m2, in_=s2, axis=AXIS.X, op=ALU.max)
            is2 = rp.tile([128, E], f32, name=f"is2{tt}", tag="is2")
            nc.vector.tensor_scalar(out=is2, in0=s2, scalar1=m2, scalar2=None, op0=ALU.is_ge)
            dm = rp.tile([128, 1], f32, name=f"dm{tt}", tag="dm")
            nc.vector.tensor_sub(dm, m2, m1)
            w2s = rp.tile([128, 1], f32, name=f"w2s{tt}", tag="w2s")
            nc.scalar.activation(out=w2s, in_=dm, func=ACTF.Sigmoid)
            # comb = is1 * (1 - w2s) + is2 * w2s
            w1s = rp.tile([128, 1], f32, name=f"w1s{tt}", tag="w1s")
            nc.scalar.activation(out=w1s, in_=w2s, func=ACTF.Identity, bias=1.0, scale=-1.0)
            comb1 = rp.tile([128, E], f32, name=f"comb1{tt}", tag="comb1")
            nc.vector.tensor_scalar(out=comb1, in0=is1, scalar1=w1s, scalar2=None, op0=ALU.mult)
            comb = rp.tile([128, E], f32, name=f"comb{tt}", tag="comb")
            nc.vector.scalar_tensor_tensor(
                out=comb, in0=is2, scalar=w2s, in1=comb1, op0=ALU.mult, op1=ALU.add,
            )
            # transpose [128, E] -> [E, 128]; keep local-expert rows
            pst = pmp.tile([E, 128], f32, name=f"pst{tt}", tag="pm")
            nc.tensor.transpose(out=pst, in_=comb, identity=identity[:, :])
            nc.vector.tensor_copy(out=combT_f[:, tsl], in_=pst[0:E_LOC, :])

        # ---- output accumulators; weighted b2 bias via K=4 matmul ----
        out_ps = []
        for hc in range(HC):
            t = pop.tile([128, T_C], f32, name=f"outp{hc}", tag=f"outp{hc}")
            out_ps.append(t)
            nc.tensor.matmul(
                out=t, lhsT=b2[0:E_LOC, hc * 128:(hc + 1) * 128], rhs=combT_f[:, :],
                start=True, stop=False,
            )

        # ---- main loop over local experts ----
        for e in range(E_LOC):
            # one big DMA per weight matrix: a single InstDMACopy is split
            # across all 16 SDMA engine slots of its queue, unlike many
            # medium DMAs which serialize at ~1 engine of bandwidth
            w1sb = w1p.tile([128, HC, F], bf16, name=f"w1_{e}", tag="w1")
            nc.sync.dma_start(
                out=w1sb, in_=w1_d[e].rearrange("(hc p) f -> p hc f", p=128))
            w2sb = w2p.tile([128, FC, H], bf16, name=f"w2_{e}", tag="w2")
            nc.scalar.dma_start(
                out=w2sb, in_=w2_d[e].rearrange("(fc p) h -> p fc h", p=128))
            w1t = [w1sb[:, hc, :] for hc in range(HC)]
            w2t = [w2sb[:, fc, :] for fc in range(FC)]

            # broadcast this expert's combine row across 128 partitions
            cb_ps = pmp.tile([128, T_C], f32, name=f"cbp{e}", tag="pm")
            nc.tensor.matmul(
                out=cb_ps, lhsT=ind[:, e * 128:(e + 1) * 128], rhs=combT_f[:, :],
                start=True, stop=True,
            )
            combB = actp.tile([128, T_C], bf16, name=f"combB{e}", tag="combB", bufs=2)
            nc.vector.tensor_copy(out=combB, in_=cb_ps)

            for fc in range(FC):
                fsl = slice(fc * 128, (fc + 1) * 128)
                hps = php.tile([128, T_C], f32, name=f"h{e}_{fc}", tag="h")
                for hc in range(HC):
                    nc.tensor.matmul(
                        out=hps, lhsT=w1t[hc][:, fsl], rhs=xTb[hc],
                        start=(hc == 0), stop=(hc == HC - 1),
                    )
                asil = actp.tile([128, T_C], bf16, name=f"as{e}_{fc}", tag="asil")
                nc.scalar.activation(
                    out=asil, in_=hps, func=ACTF.Silu,
                    bias=b1t[:, fc * E_LOC + e: fc * E_LOC + e + 1], scale=1.0,
                )
                asc = actp.tile([128, T_C], bf16, name=f"ac{e}_{fc}", tag="asc")
                nc.vector.tensor_mul(asc, asil, combB)
                for hc in range(HC):
                    nc.tensor.matmul(
                        out=out_ps[hc], lhsT=w2t[fc][:, hc * 128:(hc + 1) * 128], rhs=asc,
                        start=False, stop=(e == E_LOC - 1 and fc == FC - 1),
                    )

        # ---- epilogue: PSUM -> SBUF -> DRAM ----
        for hc in range(HC):
            ot = actp.tile([128, T_C], f32, name=f"ot{hc}", tag="ot")
            nc.vector.tensor_copy(out=ot, in_=out_ps[hc])
            nc.sync.dma_start(out=outT_d[hc * 128:(hc + 1) * 128, :], in_=ot)


def _get_nc():
    if "nc" not in _cache:
        _cache["nc"] = _build_bass()
    return _cache["nc"]


def _make_in_maps(x, Wg, W1, b1, W2, b2):
    xf = np.ascontiguousarray(x.reshape(T, H), dtype=np.float32)
    in_maps = []
    for c in range(N_CORES):
        g, h = divmod(c, 2)
        el = slice(E_LOC * h, E_LOC * (h + 1))
        perm = list(range(E_LOC * h, E_LOC * (h + 1))) + \
               [i for i in range(E) if not (E_LOC * h <= i < E_LOC * (h + 1))]
        xTc = np.ascontiguousarray(xf[g * T_C:(g + 1) * T_C].T)
        wgTc = np.ascontiguousarray(Wg[perm].T.astype(np.float32))
        w1c = np.ascontiguousarray(W1[el]).astype(ml_dtypes.bfloat16)
        w2c = np.ascontiguousarray(W2[el]).astype(ml_dtypes.bfloat16)
        b1h = np.asarray(b1[el], dtype=np.float32)
        b1tc = np.ascontiguousarray(
            b1h.reshape(E_LOC, FC, 128).transpose(2, 1, 0).reshape(128, FC * E_LOC))
        b2c = np.ascontiguousarray(b2[el], dtype=np.float32)
        indc = np.kron(np.eye(E_LOC, dtype=np.float32), np.ones((1, 128), np.float32))
        in_maps.append({
            "xT": xTc, "wgT": wgTc, "w1": w1c, "b1t": b1tc, "w2": w2c, "b2": b2c,
            "ind": indc,
        })
    return in_maps


def kernel(x, Wg, W1, b1, W2, b2, _trace=False, _trace_kwargs=None):
    from concourse.bass_utils import run_bass_kernel_spmd

    nc = _get_nc()
    in_maps = _make_in_maps(
        np.asarray(x, np.float32), np.asarray(Wg, np.float32),
        np.asarray(W1, np.float32), np.asarray(b1, np.float32),
        np.asarray(W2, np.float32), np.asarray(b2, np.float32))
    kw = {}
    if _trace:
        kw.update(trace=True, **(_trace_kwargs or {}))
    res = run_bass_kernel_spmd(nc, in_maps, core_ids=list(range(N_CORES)), **kw)
    _cache["last_results"] = res
    outs = [r["outT"] for r in res.results]
    of = np.empty((T, H), np.float32)
    for g in range(TG):
        of[g * T_C:(g + 1) * T_C] = (outs[2 * g] + outs[2 * g + 1]).T
    return of.reshape(B, S, H)
